# revision 34
# baseline (speedup 1.0000x reference)
"""Trainium2 Bass kernel for nn_BaseRGCNHetero (3-layer heterogeneous RGCN).

Strategy (8 NeuronCores, SPMD):
  - Destination-shard the nodes: core c owns rows [c*N/8, (c+1)*N/8) of every
    node type; all edges whose dst is in the shard are processed there, so
    per-relation aggregates need no cross-core reduction.
  - Aggregate-first algebra: agg[dst] = (sum_{e->dst} h[src]) @ W_r * inv_deg,
    sharing one bf16 gather table per source ntype (drug, gene) per layer.
  - After each layer the drug/gene h-shards are AllGathered (bf16) into
    per-core DRAM gather tables for the next layer.
  - Segment sums on the TensorEngine: edges are laid out as 128-slot blocks
    per (dst window, src bank).  Non-transposed dma_gather (one 256B row per
    slot, round-robined over the 4 SWDGE queues so all four Q7 core pairs
    generate descriptors in parallel) lands each block as [slot, feat] in
    SBUF; a host-precomputed one-hot [slot, dst]*inv_deg block (bf16, DMA'd
    from DRAM) is the matmul rhs; PSUM accumulates praw[feat, dst] over the
    window's blocks.  Pad slots gather row 0 and have all-zero one-hot
    columns.
  - Per window: praw -> bf16 via ScalarE, one matmul applies W_r giving the
    feature-major contribution, added into the SBUF aggregate; the self-loop
    h @ L is one more matmul; bias+relu is a fused ScalarE activation.
"""
import sys
import types
import numpy as np
import ml_dtypes
from contextlib import ExitStack

import concourse.bass as bass
import concourse.bacc as bacc
import concourse.tile as tile
from concourse import mybir, library_config

BF16 = ml_dtypes.bfloat16
P = 128
GCAP = 4096        # max slots per dma_gather
NQUEUE = 4

CFG = dict(
    N={"drug": 20000, "gene": 50000, "disease": 10000},
    MOD={"drug": 1024, "gene": 768, "disease": 512},
    D_IN=128, D_H=128, D_OUT=64,
    RELS=[("drug", "disease", "dd"), ("drug", "drug", "ddr"),
          ("drug", "gene", "dg"), ("gene", "disease", "gd"),
          ("gene", "gene", "gg")],
    NCORE=8,
    BANK=32768,     # dma_gather int16 row-index limit per table slice
)

NTYPES = ("drug", "gene", "disease")
SRC_NTYPES = ("drug", "gene")


# ---------------------------------------------------------------------------
# host-side preprocessing
# ---------------------------------------------------------------------------

def _pack_idx(stream):
    """int array (len % 128 == 0) -> dma_gather idx layout [128, len/16] int16:
    idx i at (i%16, i//16), replicated across the 8 groups of 16 partitions."""
    n = stream.size
    v = stream.astype(np.int16).reshape(n // 16, 16).T
    return np.tile(v, (8, 1))


def _banks(cfg, snt):
    n = cfg["N"][snt]
    B = cfg["BANK"]
    return [(s, min(s + B, n)) for s in range(0, n, B)]


def preprocess(cfg, inputs):
    ncore = cfg["NCORE"]
    shard = {nt: cfg["N"][nt] // ncore for nt in NTYPES}
    nw = {nt: -(-shard[nt] // P) for nt in NTYPES}

    S = dict(cfg=cfg, nw=nw, shard=shard, rels=[])
    percore = [dict() for _ in range(ncore)]

    for r, (snt, dnt, tag) in enumerate(cfg["RELS"]):
        src = np.asarray(inputs["e_" + tag + "_s"]).astype(np.int64)
        dst = np.asarray(inputs["e_" + tag + "_d"]).astype(np.int64)
        banks = _banks(cfg, snt)
        nbank = len(banks)
        NW = nw[dnt]
        dsh = shard[dnt]

        core_of = dst // dsh
        deg_all = np.bincount(dst, minlength=cfg["N"][dnt]).astype(np.int64)
        invdeg_node = (1.0 / np.maximum(deg_all, 1.0)).astype(np.float32)

        bank_of_all = src // cfg["BANK"]
        w_of_all = (dst % dsh) // P
        # per-core counts per (window, bank)
        cnt = np.zeros((ncore, NW, nbank), np.int64)
        np.add.at(cnt, (core_of, w_of_all, bank_of_all), 1)
        cmax = cnt.max(axis=0)                       # [NW, nbank]
        gsize = -(-cmax // P) * P                    # slots per (w,b), %128

        # group offsets in (bank asc, w asc) order: same-bank blocks are
        # contiguous so gathers merge up to GCAP within a bank
        goff = np.zeros((NW, nbank), np.int64)
        off = 0
        blocks = []          # (w, b, off) per 128-slot block, stream order
        for b in range(nbank):
            for w in range(NW):
                goff[w, b] = off
                for j in range(int(gsize[w, b]) // P):
                    blocks.append((w, b, off + j * P))
                off += int(gsize[w, b])
        nslots = max(off, P)

        # gathers: consecutive same-bank block runs, <= GCAP slots
        gathers = []
        cur = None
        for (w, b, boff) in blocks:
            if (cur is not None and cur[0] == b
                    and cur[2] + P <= GCAP and boff == cur[1] + cur[2]):
                cur = (b, cur[1], cur[2] + P)
            else:
                if cur is not None:
                    gathers.append(cur)
                cur = (b, boff, P)
        if cur is not None:
            gathers.append(cur)
        maxg = max(g[2] for g in gathers)

        for c in range(ncore):
            m = core_of == c
            rows = src[m]
            ld = dst[m] - c * dsh
            bnk = rows // cfg["BANK"]
            w_e = ld // P
            # position: group offset + rank within (w,b) group
            key = w_e * nbank + bnk
            so = np.argsort(key, kind="stable")
            ks = key[so]
            starts = np.r_[0, np.flatnonzero(np.diff(ks)) + 1]
            sizes = np.diff(np.r_[starts, ks.size])
            rank = np.arange(ks.size) - np.repeat(starts, sizes)
            pos = np.empty(ks.size, np.int64)
            pos[so] = goff[w_e[so], bnk[so]] + rank
            stream = np.zeros(nslots, np.int16)
            stream[pos] = (rows - bnk * cfg["BANK"]).astype(np.int16)
            percore[c][f"idx_{tag}"] = _pack_idx(stream)
            oh = np.zeros((P, nslots), np.float32)
            oh[pos % P, (pos // P) * P + (ld % P)] = invdeg_node[dst[m]]
            percore[c][f"oh_{tag}"] = oh.astype(BF16)

        S["rels"].append(dict(r=r, snt=snt, dnt=dnt, tag=tag, NW=NW,
                              banks=banks, blocks=blocks, gathers=gathers,
                              nslots=nslots, maxg=maxg))

    for nt in NTYPES:
        x = np.asarray(inputs["x_" + nt])
        for c in range(ncore):
            sh = shard[nt]
            percore[c][f"xT_{nt}"] = np.ascontiguousarray(
                x[c * sh:(c + 1) * sh].T).astype(BF16)

    com = dict()
    for nt in NTYPES:
        com[f"We_{nt}"] = np.asarray(inputs["We_" + nt]).astype(BF16)
        com[f"be_{nt}"] = np.asarray(inputs["be_" + nt]).astype(
            np.float32).reshape(-1, 1)
    for l in range(3):
        com[f"W{l}"] = np.asarray(inputs[f"W{l}"]).astype(BF16)
        com[f"L{l}"] = np.asarray(inputs[f"L{l}"]).astype(BF16)
        com[f"b{l}"] = np.asarray(inputs[f"b{l}"]).astype(np.float32).reshape(-1, 1)
    for c in range(ncore):
        percore[c].update(com)
    return S, percore


# ---------------------------------------------------------------------------
# device program
# ---------------------------------------------------------------------------

def build(S):
    cfg = S["cfg"]
    ncore = cfg["NCORE"]
    nw, shard = S["nw"], S["shard"]
    DH, DOUT = cfg["D_H"], cfg["D_OUT"]
    NREL = len(cfg["RELS"])
    nsh_tot = sum(shard.values())
    maxg_all = max(R["maxg"] for R in S["rels"])

    nc = bacc.Bacc("TRN2", target_bir_lowering=False, debug=False,
                   num_devices=ncore, num_swdge_queues=NQUEUE)

    par = {}
    for nt in NTYPES:
        par[f"xT_{nt}"] = nc.declare_dram_parameter(
            f"xT_{nt}", [cfg["MOD"][nt], shard[nt]], mybir.dt.bfloat16, False)
        par[f"We_{nt}"] = nc.declare_dram_parameter(
            f"We_{nt}", [cfg["MOD"][nt], cfg["D_IN"]], mybir.dt.bfloat16, False)
        par[f"be_{nt}"] = nc.declare_dram_parameter(
            f"be_{nt}", [cfg["D_IN"], 1], mybir.dt.float32, False)
    for l in range(3):
        od = DOUT if l == 2 else DH
        par[f"W{l}"] = nc.declare_dram_parameter(
            f"W{l}", [NREL, DH, od], mybir.dt.bfloat16, False)
        par[f"L{l}"] = nc.declare_dram_parameter(
            f"L{l}", [DH, od], mybir.dt.bfloat16, False)
        par[f"b{l}"] = nc.declare_dram_parameter(
            f"b{l}", [od, 1], mybir.dt.float32, False)
    for R in S["rels"]:
        tg = R["tag"]
        par[f"idx_{tg}"] = nc.declare_dram_parameter(
            f"idx_{tg}", [P, R["nslots"] // 16], mybir.dt.int16, False)
        par[f"oh_{tg}"] = nc.declare_dram_parameter(
            f"oh_{tg}", [P, R["nslots"]], mybir.dt.bfloat16, False)
    out_par = nc.declare_dram_parameter("out", [nsh_tot, DOUT],
                                        mybir.dt.float32, True)

    agin, tabs = {}, {}
    for l in range(3):
        for nt in SRC_NTYPES:
            agin[(l, nt)] = nc.dram_tensor(
                f"agin{l}_{nt}", [shard[nt], DH], mybir.dt.bfloat16)
            tabs[(l, nt)] = nc.dram_tensor(
                f"tab{l}_{nt}", [cfg["N"][nt], DH], mybir.dt.bfloat16,
                addr_space="Shared")

    with ExitStack() as ctx:
        tc = ctx.enter_context(tile.TileContext(nc))
        nc.gpsimd.load_library(library_config.mlp)

        const = ctx.enter_context(tc.tile_pool(name="const", bufs=1))
        persist = ctx.enter_context(tc.tile_pool(name="persist", bufs=1))
        gpool = ctx.enter_context(tc.tile_pool(name="gpool", bufs=7))
        opool = ctx.enter_context(tc.tile_pool(name="opool", bufs=5))
        ipool = ctx.enter_context(tc.tile_pool(name="ipool", bufs=12))
        xpool = ctx.enter_context(tc.tile_pool(name="xpool", bufs=2))
        wpool = ctx.enter_context(tc.tile_pool(name="wpool", bufs=4))
        pst = ctx.enter_context(tc.tile_pool(name="pst", bufs=2, space="PSUM"))
        ppr = ctx.enter_context(tc.tile_pool(name="ppr", bufs=2, space="PSUM"))
        ps1 = ctx.enter_context(tc.tile_pool(name="ps1", bufs=2, space="PSUM"))
        psE = ctx.enter_context(tc.tile_pool(name="psE", bufs=2, space="PSUM"))

        identity = const.tile([P, P], mybir.dt.float32)
        from concourse.masks import make_identity
        make_identity(nc, identity[:])
        identity16 = const.tile([P, P], mybir.dt.bfloat16)
        nc.vector.tensor_copy(identity16[:], identity[:])

        sb_W, sb_L, sb_b = {}, {}, {}
        for l in range(3):
            od = DOUT if l == 2 else DH
            t = const.tile([DH, NREL, od], mybir.dt.bfloat16, tag=f"W{l}")
            nc.sync.dma_start(t[:], par[f"W{l}"][:].rearrange("r k o -> k r o"))
            sb_W[l] = t
            sb_L[l] = const.tile([DH, od], mybir.dt.bfloat16, tag=f"L{l}",
                                 name=f"L{l}")
            nc.sync.dma_start(sb_L[l][:], par[f"L{l}"][:])
            sb_b[l] = const.tile([od, 1], mybir.dt.float32, tag=f"b{l}",
                                 name=f"b{l}")
            nc.sync.dma_start(sb_b[l][:], par[f"b{l}"][:])

        hT = [persist.tile([DH, nsh_tot], mybir.dt.bfloat16, tag=f"hT{i}",
                           name=f"hT{i}")
              for i in range(2)]
        nw_max = max(nw[nt] for nt in NTYPES)
        prall = persist.tile([P, nw_max * P], mybir.dt.bfloat16, tag="prall")
        nt_off, o = {}, 0
        for nt in NTYPES:
            nt_off[nt] = o
            o += shard[nt]
        agg = persist.tile([DH, nsh_tot], mybir.dt.bfloat16, tag="agg")

        def emit_embedding(nts):
            for nt in nts:
                mod, sh = cfg["MOD"][nt], shard[nt]
                kt = mod // P
                sb_we = xpool.tile([P, 8, cfg["D_IN"]], mybir.dt.bfloat16,
                                   tag="we")
                nc.sync.dma_start(
                    sb_we[:, :kt, :],
                    par[f"We_{nt}"][:].rearrange("(k p) f -> p k f", p=P))
                sb_be = wpool.tile([cfg["D_IN"], 1], mybir.dt.float32,
                                   tag="be")
                nc.sync.dma_start(sb_be[:], par[f"be_{nt}"][:])
                for n0 in range(0, sh, 256):
                    n1 = min(n0 + 256, sh)
                    cols = n1 - n0
                    xt = xpool.tile([P, 8, 256], mybir.dt.bfloat16, tag="xt")
                    nc.sync.dma_start(
                        xt[:, :kt, :cols],
                        par[f"xT_{nt}"][:].rearrange(
                            "(k p) n -> p k n", p=P)[:, :, n0:n1])
                    pe = psE.tile([P, 256], mybir.dt.float32, tag="emb")
                    for k in range(kt):
                        nc.tensor.matmul(pe[:, :cols], sb_we[:, k, :],
                                         xt[:, k, :cols],
                                         start=(k == 0), stop=(k == kt - 1))
                    nc.scalar.activation(
                        hT[0][:, nt_off[nt] + n0:nt_off[nt] + n1],
                        pe[:, :cols],
                        mybir.ActivationFunctionType.Identity, bias=sb_be[:])

        def emit_ag(l, nt):
            sh = shard[nt]
            for w0 in range(0, sh, P):
                cols = min(P, sh - w0)
                src = hT[l % 2][:, nt_off[nt] + w0:nt_off[nt] + w0 + cols]
                pt = pst.tile([P, P], mybir.dt.bfloat16, tag="tp",
                              name="pt16")
                nc.tensor.transpose(pt[:cols, :DH], src, identity16[:])
                stg = wpool.tile([P, DH], mybir.dt.bfloat16, tag="agstg")
                nc.vector.tensor_copy(stg[:cols, :], pt[:cols, :DH])
                nc.sync.dma_start(agin[(l, nt)][w0:w0 + cols, :],
                                  stg[:cols, :])
            nc.gpsimd.collective_compute(
                "AllGather", mybir.AluOpType.bypass,
                replica_groups=[list(range(ncore))],
                ins=[agin[(l, nt)][:]],
                outs=[tabs[(l, nt)][:]],
            )

        gq = [0]

        PF = 8   # idx-load prefetch depth (gathers ahead)

        def emit_rel(l, R):
            od = DOUT if l == 2 else DH
            tg, snt, dnt, r = R["tag"], R["snt"], R["dnt"], R["r"]
            tab = tabs[(l, snt)]
            blocks, gathers = R["blocks"], R["gathers"]
            bi = 0
            praw = None
            praw_key = None
            nblk = {}
            present = {}   # w -> banks with blocks, in stream order
            for (w, b, boff) in blocks:
                nblk[(w, b)] = nblk.get((w, b), 0) + 1
                if b not in present.setdefault(w, []):
                    present[w].append(b)
            done = {}
            idx_tiles = {}

            def load_idx(gi):
                (_, goff, gslots) = gathers[gi]
                t = ipool.tile([P, maxg_all // 16], mybir.dt.int16, tag="idx")
                nc.sync.dma_start(
                    t[:, :gslots // 16],
                    par[f"idx_{tg}"][:, goff // 16:(goff + gslots) // 16])
                idx_tiles[gi] = t

            for gi in range(min(PF, len(gathers))):
                load_idx(gi)
            for gi, (b, goff, gslots) in enumerate(gathers):
                sbi = idx_tiles.pop(gi)
                gt = gpool.tile([P, maxg_all // P, P], mybir.dt.bfloat16,
                                tag="gat")
                b0, b1 = R["banks"][b]
                nc.gpsimd.dma_gather(
                    out_ap=gt[:, :gslots // P, :], in_ap=tab[b0:b1],
                    idxs_ap=sbi[:, :gslots // 16],
                    num_idxs=gslots, num_idxs_reg=gslots,
                    elem_size=DH, transpose=False,
                    single_packet=False, queue_num=gq[0] % NQUEUE)
                gq[0] += 1
                if gi + PF < len(gathers):
                    load_idx(gi + PF)
                oh = opool.tile([P, maxg_all], mybir.dt.bfloat16, tag="oh")
                nc.sync.dma_start(
                    oh[:, :gslots],
                    par[f"oh_{tg}"][:, goff:goff + gslots])
                for j in range(gslots // P):
                    (w, bb, boff) = blocks[bi]
                    bi += 1
                    if praw_key != (w, bb):
                        praw = ppr.tile([P, P], mybir.dt.float32, tag="praw")
                        praw_key = (w, bb)
                    done[(w, bb)] = done.get((w, bb), 0) + 1
                    nc.tensor.matmul(
                        praw[:], gt[:, j, :], oh[:, j * P:(j + 1) * P],
                        start=(done[(w, bb)] == 1),
                        stop=(done[(w, bb)] == nblk[(w, bb)]))
                    if done[(w, bb)] == nblk[(w, bb)]:
                        cols = min(P, shard[dnt] - w * P)
                        if bb != present[w][-1]:
                            # earlier bank: park the window sum in SBUF
                            nc.scalar.activation(
                                prall[:, w * P:w * P + cols],
                                praw[:, :cols],
                                mybir.ActivationFunctionType.Identity)
                        else:
                            praw16 = wpool.tile([P, P], mybir.dt.bfloat16,
                                                tag="praw16")
                            nc.scalar.activation(
                                praw16[:, :cols], praw[:, :cols],
                                mybir.ActivationFunctionType.Identity)
                            p1 = ps1.tile([P, P], mybir.dt.float32, tag="p1")
                            two = len(present[w]) > 1
                            if two:
                                nc.tensor.matmul(
                                    p1[:od, :cols], sb_W[l][:, r, :],
                                    prall[:, w * P:w * P + cols],
                                    start=True, stop=False)
                            nc.tensor.matmul(p1[:od, :cols],
                                             sb_W[l][:, r, :],
                                             praw16[:, :cols],
                                             start=not two, stop=True)
                            cs = nt_off[dnt] + w * P
                            nc.vector.tensor_add(agg[:od, cs:cs + cols],
                                                 agg[:od, cs:cs + cols],
                                                 p1[:od, :cols])
                        praw_key = None
            assert bi == len(blocks)

        def emit_selfloop(l, nts):
            od = DOUT if l == 2 else DH
            for nt in nts:
                sh = shard[nt]
                for w0 in range(0, sh, P):
                    cols = min(P, sh - w0)
                    cs = nt_off[nt] + w0
                    p2 = ps1.tile([P, P], mybir.dt.float32, tag="p1")
                    nc.tensor.matmul(p2[:od, :cols], sb_L[l][:],
                                     hT[l % 2][:, cs:cs + cols],
                                     start=True, stop=True)
                    nc.vector.tensor_add(agg[:od, cs:cs + cols],
                                         agg[:od, cs:cs + cols],
                                         p2[:od, :cols])
                    if l < 2:
                        nc.scalar.activation(
                            hT[(l + 1) % 2][:od, cs:cs + cols],
                            agg[:od, cs:cs + cols],
                            mybir.ActivationFunctionType.Relu, bias=sb_b[l][:])
                    else:
                        fin = wpool.tile([P, P], mybir.dt.float32, tag="fin")
                        nc.scalar.activation(
                            fin[:od, :cols], agg[:od, cs:cs + cols],
                            mybir.ActivationFunctionType.Identity,
                            bias=sb_b[l][:])
                        pt = pst.tile([P, P], mybir.dt.float32, tag="tp")
                        nc.tensor.transpose(pt[:cols, :od], fin[:od, :cols],
                                            identity[:od, :od])
                        stg = wpool.tile([P, DOUT], mybir.dt.float32,
                                         tag="ostg")
                        nc.vector.tensor_copy(stg[:cols, :], pt[:cols, :od])
                        nc.sync.dma_start(out_par[cs:cs + cols, :],
                                          stg[:cols, :])

        emit_embedding(["drug"])
        emit_ag(0, "drug")
        emit_embedding(["gene"])
        emit_ag(0, "gene")
        emit_embedding(["disease"])
        rel_by_tag = {R["tag"]: R for R in S["rels"]}
        # Layer l's gene AllGather trigger is emitted after layer l's first
        # drug-src relation (not at the end of layer l-1): the trigger waits
        # on the gene relu tail, and emitting it late keeps it from stalling
        # the GpSimd instruction stream between layers.
        for l in range(2):
            od = DH
            nc.vector.memset(agg[:od, :], 0.0)
            emit_rel(l, rel_by_tag["dd"])
            if l >= 1:
                emit_ag(l, "gene")
            for tg in ("ddr", "dg"):
                emit_rel(l, rel_by_tag[tg])
            emit_selfloop(l, ["drug"])
            emit_ag(l + 1, "drug")
            for tg in ("gd", "gg"):
                emit_rel(l, rel_by_tag[tg])
            emit_selfloop(l, ["gene", "disease"])
        # layer 2: small drug->drug relation last so the output tail after
        # the final gather is short
        nc.vector.memset(agg[:DOUT, :], 0.0)
        emit_rel(2, rel_by_tag["dd"])
        emit_ag(2, "gene")
        emit_rel(2, rel_by_tag["dg"])
        for tg in ("gd", "gg"):
            emit_rel(2, rel_by_tag[tg])
        emit_selfloop(2, ["gene", "disease"])
        emit_rel(2, rel_by_tag["ddr"])
        emit_selfloop(2, ["drug"])

    nc.compile()
    return nc


# ---------------------------------------------------------------------------
# entry point
# ---------------------------------------------------------------------------

def _install_ntff_hook():
    if "antenv.axon_hooks" in sys.modules:
        return
    mod = types.ModuleType("antenv.axon_hooks")
    mod._hook = None
    mod.set_axon_ntff_profile_hook = lambda h: setattr(mod, "_hook", h)
    mod.get_axon_ntff_profile_hook = lambda: mod._hook
    sys.modules["antenv.axon_hooks"] = mod
    try:
        import antenv
        antenv.axon_hooks = mod
        from trn_agent_boot.trn_boot import _ntff_profile_via_ctypes
        hook = _ntff_profile_via_ctypes("/opt/axon/libaxon_pjrt.so")
        if hook is not None:
            mod.set_axon_ntff_profile_hook(hook)
    except Exception:
        pass


def run(inputs, cfg=CFG, trace=False, tmpdir=None):
    S, percore = preprocess(cfg, inputs)
    nc = build(S)
    _install_ntff_hook()
    from concourse import bass_utils
    bass_utils.upload_artifacts = lambda d: d
    res = bass_utils.run_bass_kernel_spmd(
        nc, percore, list(range(cfg["NCORE"])), trace=trace, tmpdir=tmpdir,
        trace_cores=[0] if trace else None)
    ncore = cfg["NCORE"]
    shard = {nt: cfg["N"][nt] // ncore for nt in NTYPES}
    outs = []
    o = 0
    for nt in NTYPES:
        parts = [res.results[c]["out"][o:o + shard[nt]] for c in range(ncore)]
        outs.append(np.concatenate(parts, 0))
        o += shard[nt]
    full = np.concatenate(outs, 0).astype(np.float32)
    run.last_exec_time_ns = res.exec_time_ns
    return full


def kernel(**inputs):
    return run(inputs)


# revision 36
# speedup vs baseline: 1.0085x; 1.0085x over previous
"""Trainium2 Bass kernel for nn_BaseRGCNHetero (3-layer heterogeneous RGCN).

Strategy (8 NeuronCores, SPMD):
  - Destination-shard the nodes: core c owns rows [c*N/8, (c+1)*N/8) of every
    node type; all edges whose dst is in the shard are processed there, so
    per-relation aggregates need no cross-core reduction.
  - Aggregate-first algebra: agg[dst] = (sum_{e->dst} h[src]) @ W_r * inv_deg,
    sharing one bf16 gather table per source ntype (drug, gene) per layer.
  - After each layer the drug/gene h-shards are AllGathered (bf16) into
    per-core DRAM gather tables for the next layer.
  - Segment sums on the TensorEngine: edges are laid out as 128-slot blocks
    per (dst window, src bank).  Non-transposed dma_gather (one 256B row per
    slot, round-robined over the 4 SWDGE queues so all four Q7 core pairs
    generate descriptors in parallel) lands each block as [slot, feat] in
    SBUF; a host-precomputed one-hot [slot, dst]*inv_deg block (bf16, DMA'd
    from DRAM) is the matmul rhs; PSUM accumulates praw[feat, dst] over the
    window's blocks.  Pad slots gather row 0 and have all-zero one-hot
    columns.
  - Per window: praw -> bf16 via ScalarE, one matmul applies W_r giving the
    feature-major contribution, added into the SBUF aggregate; the self-loop
    h @ L is one more matmul; bias+relu is a fused ScalarE activation.
"""
import sys
import types
import numpy as np
import ml_dtypes
from contextlib import ExitStack

import concourse.bass as bass
import concourse.bacc as bacc
import concourse.tile as tile
from concourse import mybir, library_config

BF16 = ml_dtypes.bfloat16
P = 128
GCAP = 4096        # max slots per dma_gather
NQUEUE = 4

CFG = dict(
    N={"drug": 20000, "gene": 50000, "disease": 10000},
    MOD={"drug": 1024, "gene": 768, "disease": 512},
    D_IN=128, D_H=128, D_OUT=64,
    RELS=[("drug", "disease", "dd"), ("drug", "drug", "ddr"),
          ("drug", "gene", "dg"), ("gene", "disease", "gd"),
          ("gene", "gene", "gg")],
    NCORE=8,
    BANK=32768,     # dma_gather int16 row-index limit per table slice
)

NTYPES = ("drug", "gene", "disease")
SRC_NTYPES = ("drug", "gene")


# ---------------------------------------------------------------------------
# host-side preprocessing
# ---------------------------------------------------------------------------

def _pack_idx(stream):
    """int array (len % 128 == 0) -> dma_gather idx layout [128, len/16] int16:
    idx i at (i%16, i//16), replicated across the 8 groups of 16 partitions."""
    n = stream.size
    v = stream.astype(np.int16).reshape(n // 16, 16).T
    return np.tile(v, (8, 1))


def _banks(cfg, snt):
    n = cfg["N"][snt]
    B = cfg["BANK"]
    return [(s, min(s + B, n)) for s in range(0, n, B)]


def preprocess(cfg, inputs):
    ncore = cfg["NCORE"]
    shard = {nt: cfg["N"][nt] // ncore for nt in NTYPES}
    nw = {nt: -(-shard[nt] // P) for nt in NTYPES}

    S = dict(cfg=cfg, nw=nw, shard=shard, rels=[])
    percore = [dict() for _ in range(ncore)]

    for r, (snt, dnt, tag) in enumerate(cfg["RELS"]):
        src = np.asarray(inputs["e_" + tag + "_s"]).astype(np.int64)
        dst = np.asarray(inputs["e_" + tag + "_d"]).astype(np.int64)
        banks = _banks(cfg, snt)
        nbank = len(banks)
        NW = nw[dnt]
        dsh = shard[dnt]

        core_of = dst // dsh
        deg_all = np.bincount(dst, minlength=cfg["N"][dnt]).astype(np.int64)
        invdeg_node = (1.0 / np.maximum(deg_all, 1.0)).astype(np.float32)

        bank_of_all = src // cfg["BANK"]
        w_of_all = (dst % dsh) // P
        # per-core counts per (window, bank)
        cnt = np.zeros((ncore, NW, nbank), np.int64)
        np.add.at(cnt, (core_of, w_of_all, bank_of_all), 1)
        cmax = cnt.max(axis=0)                       # [NW, nbank]
        gsize = -(-cmax // P) * P                    # slots per (w,b), %128

        # group offsets in (bank asc, w asc) order: same-bank blocks are
        # contiguous so gathers merge up to GCAP within a bank
        goff = np.zeros((NW, nbank), np.int64)
        off = 0
        blocks = []          # (w, b, off) per 128-slot block, stream order
        for b in range(nbank):
            for w in range(NW):
                goff[w, b] = off
                for j in range(int(gsize[w, b]) // P):
                    blocks.append((w, b, off + j * P))
                off += int(gsize[w, b])
        nslots = max(off, P)

        # gathers: consecutive same-bank block runs, <= GCAP slots
        gathers = []
        cur = None
        for (w, b, boff) in blocks:
            if (cur is not None and cur[0] == b
                    and cur[2] + P <= GCAP and boff == cur[1] + cur[2]):
                cur = (b, cur[1], cur[2] + P)
            else:
                if cur is not None:
                    gathers.append(cur)
                cur = (b, boff, P)
        if cur is not None:
            gathers.append(cur)
        maxg = max(g[2] for g in gathers)

        for c in range(ncore):
            m = core_of == c
            rows = src[m]
            ld = dst[m] - c * dsh
            bnk = rows // cfg["BANK"]
            w_e = ld // P
            # position: group offset + rank within (w,b) group
            key = w_e * nbank + bnk
            so = np.argsort(key, kind="stable")
            ks = key[so]
            starts = np.r_[0, np.flatnonzero(np.diff(ks)) + 1]
            sizes = np.diff(np.r_[starts, ks.size])
            rank = np.arange(ks.size) - np.repeat(starts, sizes)
            pos = np.empty(ks.size, np.int64)
            pos[so] = goff[w_e[so], bnk[so]] + rank
            stream = np.zeros(nslots, np.int16)
            stream[pos] = (rows - bnk * cfg["BANK"]).astype(np.int16)
            percore[c][f"idx_{tag}"] = _pack_idx(stream)
            oh = np.zeros((P, nslots), np.float32)
            oh[pos % P, (pos // P) * P + (ld % P)] = invdeg_node[dst[m]]
            percore[c][f"oh_{tag}"] = oh.astype(BF16)

        S["rels"].append(dict(r=r, snt=snt, dnt=dnt, tag=tag, NW=NW,
                              banks=banks, blocks=blocks, gathers=gathers,
                              nslots=nslots, maxg=maxg))

    for nt in NTYPES:
        x = np.asarray(inputs["x_" + nt])
        for c in range(ncore):
            sh = shard[nt]
            percore[c][f"xT_{nt}"] = np.ascontiguousarray(
                x[c * sh:(c + 1) * sh].T).astype(BF16)

    com = dict()
    for nt in NTYPES:
        com[f"We_{nt}"] = np.asarray(inputs["We_" + nt]).astype(BF16)
        com[f"be_{nt}"] = np.asarray(inputs["be_" + nt]).astype(
            np.float32).reshape(-1, 1)
    for l in range(3):
        com[f"W{l}"] = np.asarray(inputs[f"W{l}"]).astype(BF16)
        com[f"L{l}"] = np.asarray(inputs[f"L{l}"]).astype(BF16)
        com[f"b{l}"] = np.asarray(inputs[f"b{l}"]).astype(np.float32).reshape(-1, 1)
    for c in range(ncore):
        percore[c].update(com)
    return S, percore


# ---------------------------------------------------------------------------
# device program
# ---------------------------------------------------------------------------

def build(S):
    cfg = S["cfg"]
    ncore = cfg["NCORE"]
    nw, shard = S["nw"], S["shard"]
    DH, DOUT = cfg["D_H"], cfg["D_OUT"]
    NREL = len(cfg["RELS"])
    nsh_tot = sum(shard.values())
    maxg_all = max(R["maxg"] for R in S["rels"])

    nc = bacc.Bacc("TRN2", target_bir_lowering=False, debug=False,
                   num_devices=ncore, num_swdge_queues=NQUEUE)

    par = {}
    for nt in NTYPES:
        par[f"xT_{nt}"] = nc.declare_dram_parameter(
            f"xT_{nt}", [cfg["MOD"][nt], shard[nt]], mybir.dt.bfloat16, False)
        par[f"We_{nt}"] = nc.declare_dram_parameter(
            f"We_{nt}", [cfg["MOD"][nt], cfg["D_IN"]], mybir.dt.bfloat16, False)
        par[f"be_{nt}"] = nc.declare_dram_parameter(
            f"be_{nt}", [cfg["D_IN"], 1], mybir.dt.float32, False)
    for l in range(3):
        od = DOUT if l == 2 else DH
        par[f"W{l}"] = nc.declare_dram_parameter(
            f"W{l}", [NREL, DH, od], mybir.dt.bfloat16, False)
        par[f"L{l}"] = nc.declare_dram_parameter(
            f"L{l}", [DH, od], mybir.dt.bfloat16, False)
        par[f"b{l}"] = nc.declare_dram_parameter(
            f"b{l}", [od, 1], mybir.dt.float32, False)
    for R in S["rels"]:
        tg = R["tag"]
        par[f"idx_{tg}"] = nc.declare_dram_parameter(
            f"idx_{tg}", [P, R["nslots"] // 16], mybir.dt.int16, False)
        par[f"oh_{tg}"] = nc.declare_dram_parameter(
            f"oh_{tg}", [P, R["nslots"]], mybir.dt.bfloat16, False)
    out_par = nc.declare_dram_parameter("out", [nsh_tot, DOUT],
                                        mybir.dt.float32, True)

    agin, tabs = {}, {}
    for l in range(3):
        for nt in SRC_NTYPES:
            agin[(l, nt)] = nc.dram_tensor(
                f"agin{l}_{nt}", [shard[nt], DH], mybir.dt.bfloat16)
            tabs[(l, nt)] = nc.dram_tensor(
                f"tab{l}_{nt}", [cfg["N"][nt], DH], mybir.dt.bfloat16,
                addr_space="Shared")

    with ExitStack() as ctx:
        tc = ctx.enter_context(tile.TileContext(nc))
        nc.gpsimd.load_library(library_config.mlp)

        const = ctx.enter_context(tc.tile_pool(name="const", bufs=1))
        persist = ctx.enter_context(tc.tile_pool(name="persist", bufs=1))
        gpool = ctx.enter_context(tc.tile_pool(name="gpool", bufs=8))
        opool = ctx.enter_context(tc.tile_pool(name="opool", bufs=4))
        ipool = ctx.enter_context(tc.tile_pool(name="ipool", bufs=12))
        xpool = ctx.enter_context(tc.tile_pool(name="xpool", bufs=2))
        wpool = ctx.enter_context(tc.tile_pool(name="wpool", bufs=4))
        pst = ctx.enter_context(tc.tile_pool(name="pst", bufs=2, space="PSUM"))
        ppr = ctx.enter_context(tc.tile_pool(name="ppr", bufs=2, space="PSUM"))
        ps1 = ctx.enter_context(tc.tile_pool(name="ps1", bufs=2, space="PSUM"))
        psE = ctx.enter_context(tc.tile_pool(name="psE", bufs=2, space="PSUM"))

        identity = const.tile([P, P], mybir.dt.float32)
        from concourse.masks import make_identity
        make_identity(nc, identity[:])
        identity16 = const.tile([P, P], mybir.dt.bfloat16)
        nc.vector.tensor_copy(identity16[:], identity[:])

        sb_W, sb_L, sb_b = {}, {}, {}
        for l in range(3):
            od = DOUT if l == 2 else DH
            t = const.tile([DH, NREL, od], mybir.dt.bfloat16, tag=f"W{l}")
            nc.sync.dma_start(t[:], par[f"W{l}"][:].rearrange("r k o -> k r o"))
            sb_W[l] = t
            sb_L[l] = const.tile([DH, od], mybir.dt.bfloat16, tag=f"L{l}",
                                 name=f"L{l}")
            nc.sync.dma_start(sb_L[l][:], par[f"L{l}"][:])
            sb_b[l] = const.tile([od, 1], mybir.dt.float32, tag=f"b{l}",
                                 name=f"b{l}")
            nc.sync.dma_start(sb_b[l][:], par[f"b{l}"][:])

        hT = [persist.tile([DH, nsh_tot], mybir.dt.bfloat16, tag=f"hT{i}",
                           name=f"hT{i}")
              for i in range(2)]
        nw_max = max(nw[nt] for nt in NTYPES)
        prall = persist.tile([P, nw_max * P], mybir.dt.bfloat16, tag="prall")
        nt_off, o = {}, 0
        for nt in NTYPES:
            nt_off[nt] = o
            o += shard[nt]
        agg = persist.tile([DH, nsh_tot], mybir.dt.bfloat16, tag="agg")

        def emit_embedding(nts):
            for nt in nts:
                mod, sh = cfg["MOD"][nt], shard[nt]
                kt = mod // P
                sb_we = xpool.tile([P, 8, cfg["D_IN"]], mybir.dt.bfloat16,
                                   tag="we")
                nc.sync.dma_start(
                    sb_we[:, :kt, :],
                    par[f"We_{nt}"][:].rearrange("(k p) f -> p k f", p=P))
                sb_be = wpool.tile([cfg["D_IN"], 1], mybir.dt.float32,
                                   tag="be")
                nc.sync.dma_start(sb_be[:], par[f"be_{nt}"][:])
                for n0 in range(0, sh, 256):
                    n1 = min(n0 + 256, sh)
                    cols = n1 - n0
                    xt = xpool.tile([P, 8, 256], mybir.dt.bfloat16, tag="xt")
                    nc.sync.dma_start(
                        xt[:, :kt, :cols],
                        par[f"xT_{nt}"][:].rearrange(
                            "(k p) n -> p k n", p=P)[:, :, n0:n1])
                    pe = psE.tile([P, 256], mybir.dt.float32, tag="emb")
                    for k in range(kt):
                        nc.tensor.matmul(pe[:, :cols], sb_we[:, k, :],
                                         xt[:, k, :cols],
                                         start=(k == 0), stop=(k == kt - 1))
                    nc.scalar.activation(
                        hT[0][:, nt_off[nt] + n0:nt_off[nt] + n1],
                        pe[:, :cols],
                        mybir.ActivationFunctionType.Identity, bias=sb_be[:])

        def emit_ag(l, nt):
            sh = shard[nt]
            for w0 in range(0, sh, P):
                cols = min(P, sh - w0)
                src = hT[l % 2][:, nt_off[nt] + w0:nt_off[nt] + w0 + cols]
                pt = pst.tile([P, P], mybir.dt.bfloat16, tag="tp",
                              name="pt16")
                nc.tensor.transpose(pt[:cols, :DH], src, identity16[:])
                stg = wpool.tile([P, DH], mybir.dt.bfloat16, tag="agstg")
                nc.vector.tensor_copy(stg[:cols, :], pt[:cols, :DH])
                nc.sync.dma_start(agin[(l, nt)][w0:w0 + cols, :],
                                  stg[:cols, :])
            nc.gpsimd.collective_compute(
                "AllGather", mybir.AluOpType.bypass,
                replica_groups=[list(range(ncore))],
                ins=[agin[(l, nt)][:]],
                outs=[tabs[(l, nt)][:]],
            )

        gq = [0]

        PF = 8   # idx-load prefetch depth (gathers ahead)

        def emit_rel(l, R):
            od = DOUT if l == 2 else DH
            tg, snt, dnt, r = R["tag"], R["snt"], R["dnt"], R["r"]
            tab = tabs[(l, snt)]
            blocks, gathers = R["blocks"], R["gathers"]
            bi = 0
            praw = None
            praw_key = None
            nblk = {}
            present = {}   # w -> banks with blocks, in stream order
            for (w, b, boff) in blocks:
                nblk[(w, b)] = nblk.get((w, b), 0) + 1
                if b not in present.setdefault(w, []):
                    present[w].append(b)
            done = {}
            idx_tiles = {}

            def load_idx(gi):
                (_, goff, gslots) = gathers[gi]
                t = ipool.tile([P, maxg_all // 16], mybir.dt.int16, tag="idx")
                nc.sync.dma_start(
                    t[:, :gslots // 16],
                    par[f"idx_{tg}"][:, goff // 16:(goff + gslots) // 16])
                idx_tiles[gi] = t

            for gi in range(min(PF, len(gathers))):
                load_idx(gi)
            for gi, (b, goff, gslots) in enumerate(gathers):
                sbi = idx_tiles.pop(gi)
                gt = gpool.tile([P, maxg_all // P, P], mybir.dt.bfloat16,
                                tag="gat")
                b0, b1 = R["banks"][b]
                nc.gpsimd.dma_gather(
                    out_ap=gt[:, :gslots // P, :], in_ap=tab[b0:b1],
                    idxs_ap=sbi[:, :gslots // 16],
                    num_idxs=gslots, num_idxs_reg=gslots,
                    elem_size=DH, transpose=False,
                    single_packet=False, queue_num=gq[0] % NQUEUE)
                gq[0] += 1
                if gi + PF < len(gathers):
                    load_idx(gi + PF)
                oh = opool.tile([P, maxg_all], mybir.dt.bfloat16, tag="oh")
                nc.sync.dma_start(
                    oh[:, :gslots],
                    par[f"oh_{tg}"][:, goff:goff + gslots])
                for j in range(gslots // P):
                    (w, bb, boff) = blocks[bi]
                    bi += 1
                    if praw_key != (w, bb):
                        praw = ppr.tile([P, P], mybir.dt.float32, tag="praw")
                        praw_key = (w, bb)
                    done[(w, bb)] = done.get((w, bb), 0) + 1
                    nc.tensor.matmul(
                        praw[:], gt[:, j, :], oh[:, j * P:(j + 1) * P],
                        start=(done[(w, bb)] == 1),
                        stop=(done[(w, bb)] == nblk[(w, bb)]))
                    if done[(w, bb)] == nblk[(w, bb)]:
                        cols = min(P, shard[dnt] - w * P)
                        if bb != present[w][-1]:
                            # earlier bank: park the window sum in SBUF
                            nc.scalar.activation(
                                prall[:, w * P:w * P + cols],
                                praw[:, :cols],
                                mybir.ActivationFunctionType.Identity)
                        else:
                            praw16 = wpool.tile([P, P], mybir.dt.bfloat16,
                                                tag="praw16")
                            nc.scalar.activation(
                                praw16[:, :cols], praw[:, :cols],
                                mybir.ActivationFunctionType.Identity)
                            p1 = ps1.tile([P, P], mybir.dt.float32, tag="p1")
                            two = len(present[w]) > 1
                            if two:
                                nc.tensor.matmul(
                                    p1[:od, :cols], sb_W[l][:, r, :],
                                    prall[:, w * P:w * P + cols],
                                    start=True, stop=False)
                            nc.tensor.matmul(p1[:od, :cols],
                                             sb_W[l][:, r, :],
                                             praw16[:, :cols],
                                             start=not two, stop=True)
                            cs = nt_off[dnt] + w * P
                            nc.vector.tensor_add(agg[:od, cs:cs + cols],
                                                 agg[:od, cs:cs + cols],
                                                 p1[:od, :cols])
                        praw_key = None
            assert bi == len(blocks)

        def emit_selfloop(l, nts):
            od = DOUT if l == 2 else DH
            for nt in nts:
                sh = shard[nt]
                for w0 in range(0, sh, P):
                    cols = min(P, sh - w0)
                    cs = nt_off[nt] + w0
                    p2 = ps1.tile([P, P], mybir.dt.float32, tag="p1")
                    nc.tensor.matmul(p2[:od, :cols], sb_L[l][:],
                                     hT[l % 2][:, cs:cs + cols],
                                     start=True, stop=True)
                    nc.vector.tensor_add(agg[:od, cs:cs + cols],
                                         agg[:od, cs:cs + cols],
                                         p2[:od, :cols])
                    if l < 2:
                        nc.scalar.activation(
                            hT[(l + 1) % 2][:od, cs:cs + cols],
                            agg[:od, cs:cs + cols],
                            mybir.ActivationFunctionType.Relu, bias=sb_b[l][:])
                    else:
                        fin = wpool.tile([P, P], mybir.dt.float32, tag="fin")
                        nc.scalar.activation(
                            fin[:od, :cols], agg[:od, cs:cs + cols],
                            mybir.ActivationFunctionType.Identity,
                            bias=sb_b[l][:])
                        pt = pst.tile([P, P], mybir.dt.float32, tag="tp")
                        nc.tensor.transpose(pt[:cols, :od], fin[:od, :cols],
                                            identity[:od, :od])
                        stg = wpool.tile([P, DOUT], mybir.dt.float32,
                                         tag="ostg")
                        nc.vector.tensor_copy(stg[:cols, :], pt[:cols, :od])
                        nc.sync.dma_start(out_par[cs:cs + cols, :],
                                          stg[:cols, :])

        emit_embedding(["drug"])
        emit_ag(0, "drug")
        emit_embedding(["gene"])
        emit_ag(0, "gene")
        emit_embedding(["disease"])
        rel_by_tag = {R["tag"]: R for R in S["rels"]}
        for l in range(2):
            od = DH
            nc.vector.memset(agg[:od, :], 0.0)
            for tg in ("dd", "ddr", "dg"):
                emit_rel(l, rel_by_tag[tg])
            emit_selfloop(l, ["drug"])
            emit_ag(l + 1, "drug")
            for tg in ("gd", "gg"):
                emit_rel(l, rel_by_tag[tg])
            emit_selfloop(l, ["gene", "disease"])
            emit_ag(l + 1, "gene")
        # layer 2: put the small drug->drug relation last so the output tail
        # after the final gather is short
        nc.vector.memset(agg[:DOUT, :], 0.0)
        for tg in ("dd", "dg", "gd", "gg"):
            emit_rel(2, rel_by_tag[tg])
        emit_selfloop(2, ["gene", "disease"])
        emit_rel(2, rel_by_tag["ddr"])
        emit_selfloop(2, ["drug"])

    nc.compile()
    return nc


# ---------------------------------------------------------------------------
# entry point
# ---------------------------------------------------------------------------

def _install_ntff_hook():
    if "antenv.axon_hooks" in sys.modules:
        return
    mod = types.ModuleType("antenv.axon_hooks")
    mod._hook = None
    mod.set_axon_ntff_profile_hook = lambda h: setattr(mod, "_hook", h)
    mod.get_axon_ntff_profile_hook = lambda: mod._hook
    sys.modules["antenv.axon_hooks"] = mod
    try:
        import antenv
        antenv.axon_hooks = mod
        from trn_agent_boot.trn_boot import _ntff_profile_via_ctypes
        hook = _ntff_profile_via_ctypes("/opt/axon/libaxon_pjrt.so")
        if hook is not None:
            mod.set_axon_ntff_profile_hook(hook)
    except Exception:
        pass


def run(inputs, cfg=CFG, trace=False, tmpdir=None):
    S, percore = preprocess(cfg, inputs)
    nc = build(S)
    _install_ntff_hook()
    from concourse import bass_utils
    bass_utils.upload_artifacts = lambda d: d
    res = bass_utils.run_bass_kernel_spmd(
        nc, percore, list(range(cfg["NCORE"])), trace=trace, tmpdir=tmpdir,
        trace_cores=[0] if trace else None)
    ncore = cfg["NCORE"]
    shard = {nt: cfg["N"][nt] // ncore for nt in NTYPES}
    outs = []
    o = 0
    for nt in NTYPES:
        parts = [res.results[c]["out"][o:o + shard[nt]] for c in range(ncore)]
        outs.append(np.concatenate(parts, 0))
        o += shard[nt]
    full = np.concatenate(outs, 0).astype(np.float32)
    run.last_exec_time_ns = res.exec_time_ns
    return full


def kernel(**inputs):
    return run(inputs)
